# revision 32
# baseline (speedup 1.0000x reference)
"""Trainium2 Bass kernel for nn_BoundaryLoss (retrieval 1-NN + boundary loss).

Math reformulation (validated against the reference on the fixed inputs):
rigid SE(3) transforms preserve distances and dot products, so the 1-NN
search and the signed-distance dot product can both be done in the GLOBAL
frame.  With wg = R_b @ w + t_b (waypoints to global frame, tiny host prep),
the per-(b,t) argmin over boundary points n of |w_local - p_local|^2 equals
argmax_n s'[n],  s'[n] = 2*wg.pg[n] - |pg[n]|^2,
and dots = (w_local - cp).cn = wg.ng[idx] - pg[idx].ng[idx].

Device pipeline per core (8-way data parallel over the 6400 (b,t) pairs),
per 128-waypoint tile (7 tiles per core):
  - PE: s'/8 via K=11 fp16 hi/lo split matmuls (a*b = ah*bh + ah*bl + al*bh
        per coordinate + 2 rows for p^2/8), fp32 PSUM accumulation; 4
        matmuls per 4-bank PSUM group, 2 groups ping-ponging.
  - ACT (+1 group on DVE): ONE 2048-wide PSUM->SBUF fp16 read per group.
        Multi-bank engine reads are safe because the Tile overlap tracker
        is bank-aware: no matmul writes those banks concurrently (the
        fatal case is PE-W || engine-R on the same bank).
  - DVE: one pairwise max of the halves (fp16 tensor_tensor), then ONE
        custom single-pass argmax instruction that pairs the quarters
        itself and scans them (ARGMAX_LAST_ANT: p = maxx(Src0, Src1);
        body = select(p == scan_max(p), Idx/16, -FLT_MAX), accum = MAX)
        over the 5120-wide quarter row.  Winner k gives the candidate
        set {k, k+5120, k+10240, k+15360}.
  - GPSIMD: four indirect-DMA gathers of [pg, p2, ng, pg.ng] rows per tile.
  - Tail (once, batched over [128, 7]): exact-fp32 rescore of the four
        candidates, select-tree pick, dots, exp_relu, mask; ones-matmul
        partition reduction -> [1, 7] per-core partial sums.
The scoring of tile j is software-pipelined with the argmax of tile j-1 so
DVE's evacuation copies never queue behind a scan, and the rhs table DMA is
split per group so the first matmul starts early.
Host: input prep/sharding + final sum of 8x7 partials / 6400.
Validated numerically on the fixed harness inputs: loss rel err 4.2e-05
(gate 2e-2).

HW notes (measured on the target cores): engine PSUM reads must not
overlap a concurrent matmul write to the same bank (Tile guards this);
matmul moving operand is capped at 512 fp32 PSUM columns (s3d3 ISA check
rejects 1024 for fp16); float32r matmul quantizes inputs to ~13 mantissa
bits (argmax-fatal); custom DVE ops stream at ~1.5 cyc/elem regardless of
spec depth; stock fp16 unit-stride tensor_tensor hits the 2x mode.
"""

import sys

sys.path.insert(0, "/opt/trn_rl_repo")

import numpy as np

from concourse import bacc, bass, mybir
import concourse.tile as tile
from concourse.bass_utils import run_bass_kernel_spmd
from concourse.dve_spec import (Spec, Src0, Src1, C2, MaxNeg, select, eq,
                                lower, AluOp, Idx, scan, maxx)
from concourse.dve_uop import DveOpSpec
import concourse.dve_ops as dve_ops
from concourse.dve_ops import DveOp

B, T, N = 64, 100, 20000
NCORES = 8
WPC = B * T // NCORES          # 800 waypoints per core
NTILES = 7                     # ceil(WPC / 128) partition tiles
CHUNK = 512                    # one PSUM bank of fp32
GRP = 2048                     # grouped evacuation width (4 banks)
NGRP = 10                      # groups per boundary row
DVE_GROUPS = (0, 2)            # groups evacuated by DVE at tile start,
                               # before the pipelined argmax occupies the
                               # engine (rest on ACT; measured optimum --
                               # any later DVE evac position regresses)
NCH = 40                       # 512-chunks per boundary row
NPAD = NCH * CHUNK             # 20480
SEG = NPAD // 2                # first pairing half width (10240)
QRT = NPAD // 4                # second pairing width (5120)
WPAD = NTILES * 128            # 896
KSPLIT = 11                    # fp16 split-matmul contraction rows

F32 = mybir.dt.float32
F16 = mybir.dt.float16
U32 = mybir.dt.uint32
U8 = mybir.dt.uint8
OP = mybir.AluOpType
AX = mybir.AxisListType
AF = mybir.ActivationFunctionType

# --- custom DVE op: single-pass last-tie-wins argmax (index scaled by 1/16
# so the fold stays exact even if the accumulator ran on post-cast fp16) ---
IDX_SCALE = 1.0 / 16.0
_p = maxx(Src0, Src1)
_ARGMAX_SPEC = Spec(body=select(eq(_p, scan(AluOp.MAX, _p)), Idx * C2, MaxNeg),
                    accum=AluOp.MAX)


def _register_argmax_op():
    name = "ARGMAX_LAST_ANT"
    for op in dve_ops.OPS:
        if op.name == name:
            return op

    def sha(ver):
        return DveOpSpec(name="tmp", opcode=1,
                         uops=lower(_ARGMAX_SPEC, ver=ver),
                         rd1_en=True).sha(ver)

    op = DveOp(name, _ARGMAX_SPEC, subdim=False,
               uops_sha={v: sha(v) for v in ("v3", "v4")})
    dve_ops.OPS.append(op)
    dve_ops.CUSTOM_DVE_SPECS[name] = _ARGMAX_SPEC
    row = max(dve_ops._SUB_OPCODE_FOR_NAME.values()) + 1
    assert row < 0x20
    dve_ops._SUB_OPCODE_FOR_NAME[name] = row
    return op


ARGMAX_LAST = _register_argmax_op()


def build(repeat=1):
    nc = bacc.Bacc("TRN2", target_bir_lowering=False, debug=False,
                   num_devices=NCORES)
    lhs = nc.dram_tensor("lhs", [KSPLIT, WPAD], F16, kind="ExternalInput").ap()
    rhs = nc.dram_tensor("rhs", [KSPLIT, NPAD], F16, kind="ExternalInput").ap()
    wgv = nc.dram_tensor("wgv", [128, NTILES, 3], F32, kind="ExternalInput").ap()
    msk = nc.dram_tensor("msk", [128, NTILES], F32, kind="ExternalInput").ap()
    tb8 = nc.dram_tensor("tb8", [NPAD, 8], F32, kind="ExternalInput").ap()
    out = nc.dram_tensor("out", [1, NTILES], F32, kind="ExternalOutput").ap()

    with tile.TileContext(nc) as tc:
        with (
            tc.tile_pool(name="const", bufs=1) as cpool,
            tc.tile_pool(name="s16p", bufs=2) as s16p,
            tc.tile_pool(name="mp", bufs=2) as mp,
            tc.tile_pool(name="sb", bufs=3) as sb,
            tc.tile_pool(name="ps", bufs=2, space="PSUM") as ps,
        ):
            lhs_sb = cpool.tile([KSPLIT, WPAD], F16)
            nc.sync.dma_start(out=lhs_sb[:], in_=lhs[:])
            rhs_sb = cpool.tile([KSPLIT, NPAD], F16)
            for g in range(NGRP):
                nc.sync.dma_start(out=rhs_sb[:, g * GRP:(g + 1) * GRP],
                                  in_=rhs[:, g * GRP:(g + 1) * GRP])
            wgv_sb = cpool.tile([128, NTILES, 3], F32)
            nc.sync.dma_start(out=wgv_sb[:], in_=wgv[:])
            msk_sb = cpool.tile([128, NTILES], F32)
            nc.sync.dma_start(out=msk_sb[:], in_=msk[:])
            ones_sb = cpool.tile([128, 1], F32)
            nc.vector.memset(ones_sb[:], 1.0)
            cand = cpool.tile([128, NTILES, 4, 8], F32)
            am_all = cpool.tile([128, NTILES], F32)

            s16s = {}

            def emit_tile(j):
                s16 = s16p.tile([128, NPAD], F16, tag="s16")
                s16s[j] = s16
                for g in range(NGRP):
                    pg = ps.tile([128, GRP], F32, tag="mm")
                    for k in range(4):
                        c = 4 * g + k
                        nc.tensor.matmul(
                            out=pg[:, k * CHUNK:(k + 1) * CHUNK],
                            lhsT=lhs_sb[:, j * 128:(j + 1) * 128],
                            rhs=rhs_sb[:, c * CHUNK:(c + 1) * CHUNK],
                            start=True, stop=True,
                        )
                    dst = s16[:, g * GRP:(g + 1) * GRP]
                    if g in DVE_GROUPS:
                        nc.vector.tensor_copy(dst, pg[:])
                    else:
                        nc.scalar.activation(dst, pg[:], AF.Copy)

            def emit_proc(j):
                # two fp16 2x pairing rounds, then one-pass argmax over 5120
                s16 = s16s.pop(j)
                m = mp.tile([128, SEG], F16, tag="m")
                nc.vector.tensor_tensor(out=m[:], in0=s16[:, 0:SEG],
                                        in1=s16[:, SEG:NPAD], op=OP.max)
                # argmax op pairs the quarters itself (maxx(Src0, Src1))
                nc.vector._custom_dve(ARGMAX_LAST, out=m[:, 0:QRT],
                                      in0=m[:, 0:QRT], in1=m[:, QRT:SEG],
                                      imm2=IDX_SCALE,
                                      accum_out=am_all[:, j:j + 1])
                idxu = sb.tile([128, 4], U32, tag="idxu")
                for c in range(4):
                    nc.vector.tensor_scalar(idxu[:, c:c + 1],
                                            am_all[:, j:j + 1],
                                            1.0 / IDX_SCALE, float(c * QRT),
                                            OP.mult, OP.add)
                # gather [pg, p2, ng, pg.ng] rows for all four candidates
                for c in range(4):
                    nc.gpsimd.indirect_dma_start(
                        out=cand[:, j, c, :], out_offset=None, in_=tb8[:],
                        in_offset=bass.IndirectOffsetOnAxis(
                            ap=idxu[:, c:c + 1], axis=0),
                    )

            # software pipeline: tile j-1's argmax is emitted BEFORE tile
            # j's scoring so it runs on DVE while ACT drains the early
            # groups, and DVE's own (late-positioned) evacuation copies
            # are not queued behind the scan
            for j in range(NTILES * repeat):
                emit_tile(j % NTILES)
                if j > 0:
                    emit_proc((j - 1) % NTILES)
            emit_proc((NTILES * repeat - 1) % NTILES)

            # batched tail over [128, NTILES]: exact rescore, pick tree,
            # dots, exp_relu, mask
            sc = sb.tile([128, 4, NTILES], F32, tag="sc")
            dt = sb.tile([128, 4, NTILES], F32, tag="dt")
            t3 = sb.tile([128, NTILES, 3], F32, tag="t3")
            tr = sb.tile([128, NTILES], F32, tag="tr")
            for c in range(4):
                nc.vector.tensor_tensor(out=t3[:], in0=wgv_sb[:],
                                        in1=cand[:, :, c, 0:3], op=OP.mult)
                nc.vector.tensor_reduce(out=tr[:], in_=t3[:], axis=AX.X,
                                        op=OP.add)
                nc.vector.scalar_tensor_tensor(
                    out=sc[:, c, :], in0=tr[:], scalar=2.0,
                    in1=cand[:, :, c, 3], op0=OP.mult, op1=OP.subtract)
                nc.vector.tensor_tensor(out=t3[:], in0=wgv_sb[:],
                                        in1=cand[:, :, c, 4:7], op=OP.mult)
                nc.vector.tensor_reduce(out=tr[:], in_=t3[:], axis=AX.X,
                                        op=OP.add)
                nc.vector.tensor_tensor(out=dt[:, c, :], in0=tr[:],
                                        in1=cand[:, :, c, 7], op=OP.subtract)
            ge = sb.tile([128, NTILES], U8, tag="ge")
            sw = sb.tile([128, 2, NTILES], F32, tag="sw")
            dw = sb.tile([128, 2, NTILES], F32, tag="dw")
            for h in range(2):
                nc.vector.tensor_tensor(out=ge[:], in0=sc[:, 2 * h, :],
                                        in1=sc[:, 2 * h + 1, :], op=OP.is_ge)
                nc.vector.select(sw[:, h, :], ge[:], sc[:, 2 * h, :],
                                 sc[:, 2 * h + 1, :])
                nc.vector.select(dw[:, h, :], ge[:], dt[:, 2 * h, :],
                                 dt[:, 2 * h + 1, :])
            nc.vector.tensor_tensor(out=ge[:], in0=sw[:, 0, :],
                                    in1=sw[:, 1, :], op=OP.is_ge)
            dots = sb.tile([128, NTILES], F32, tag="dots")
            nc.vector.select(dots[:], ge[:], dw[:, 0, :], dw[:, 1, :])

            ecl = sb.tile([128, NTILES], F32, tag="ecl")
            nc.vector.tensor_scalar_min(ecl[:], dots[:], 0.0)
            ex = sb.tile([128, NTILES], F32, tag="ex")
            nc.scalar.activation(ex[:], ecl[:], AF.Exp, scale=0.5)
            p1 = sb.tile([128, NTILES], F32, tag="p1")
            nc.vector.tensor_scalar_add(p1[:], dots[:], 1.0)
            gt = sb.tile([128, NTILES], U8, tag="gt")
            nc.vector.tensor_scalar(gt[:], dots[:], 0.0, None, OP.is_gt)
            er = sb.tile([128, NTILES], F32, tag="er")
            nc.vector.select(er[:], gt[:], p1[:], ex[:])
            erm = sb.tile([128, NTILES], F32, tag="erm")
            nc.vector.tensor_tensor(out=erm[:], in0=er[:], in1=msk_sb[:],
                                    op=OP.mult)

            po = ps.tile([1, NTILES], F32, tag="mm")
            nc.tensor.matmul(out=po[:], lhsT=ones_sb[:, 0:1], rhs=erm[:],
                             start=True, stop=True)
            ob = sb.tile([1, NTILES], F32, tag="ob")
            nc.vector.tensor_copy(ob[:], po[:])
            nc.sync.dma_start(out=out[:], in_=ob[:])

    nc.compile()
    return nc


def _f16_split(x32):
    hi = x32.astype(np.float16)
    lo = (x32 - hi.astype(np.float32)).astype(np.float16)
    return hi, lo


def prep_inputs(posesglobal, waypointslocal, boundary, boundarynormals):
    poses = np.asarray(posesglobal, dtype=np.float32)
    wpts = np.asarray(waypointslocal, dtype=np.float32)
    bound = np.asarray(boundary, dtype=np.float32)
    nrm = np.asarray(boundarynormals, dtype=np.float32)

    R = poses[:, :3, :3]
    t = poses[:, :3, 3]
    wg = (np.einsum("bij,btj->bti", R, wpts).astype(np.float32)
          + t[:, None, :]).astype(np.float32).reshape(-1, 3)   # [B*T, 3]

    pg = bound[:3]
    p2 = (pg[0] * pg[0] + pg[1] * pg[1] + pg[2] * pg[2]).astype(np.float32)
    pn = (pg[0] * nrm[0] + pg[1] * nrm[1] + pg[2] * nrm[2]).astype(np.float32)

    # rhs rows: per coord d -> [bh_d, bl_d, bh_d]; then [ch, cl] for p2/8
    bh, bl = _f16_split(pg)                     # [3, N] each
    ch, cl = _f16_split(p2 / 8.0)
    rhs = np.zeros((KSPLIT, NPAD), np.float16)
    for d in range(3):
        rhs[3 * d + 0, :N] = bh[d]
        rhs[3 * d + 1, :N] = bl[d]
        rhs[3 * d + 2, :N] = bh[d]
    rhs[9, :N] = ch
    rhs[10, :N] = cl
    rhs[9, N:] = np.float16(60000.0)   # pad columns can never win the argmax

    # combined gather table [pg, p2, ng, pg.ng]; pad rows lose the exact
    # rescore via p2 = 1e30 (candidate 1 may index into the pad range)
    tb8 = np.zeros((NPAD, 8), np.float32)
    tb8[:N, 0:3] = pg.T
    tb8[:N, 3] = p2
    tb8[:N, 4:7] = nrm.T
    tb8[:N, 7] = pn
    tb8[N:, 3] = 1.0e30

    valid = (np.arange(WPAD) < WPC)
    msk = valid.reshape(NTILES, 128).T.astype(np.float32).copy()  # [128, 7]

    in_maps = []
    for c in range(NCORES):
        w = wg[c * WPC:(c + 1) * WPC]
        wp = np.zeros((WPAD, 3), np.float32)
        wp[:WPC] = w
        ah, al = _f16_split(wp.T / 4.0)          # [3, WPAD] each (= 2*wg/8)
        lhs = np.zeros((KSPLIT, WPAD), np.float16)
        for d in range(3):
            lhs[3 * d + 0] = ah[d]
            lhs[3 * d + 1] = ah[d]
            lhs[3 * d + 2] = al[d]
        lhs[9] = np.float16(-1.0)
        lhs[10] = np.float16(-1.0)
        wgv = wp.reshape(NTILES, 128, 3).transpose(1, 0, 2).copy()
        in_maps.append({"lhs": lhs, "rhs": rhs, "wgv": wgv,
                        "msk": msk, "tb8": tb8})
    return in_maps


_CACHE = {}


def kernel(posesglobal, waypointslocal, boundary, boundarynormals):
    if "nc" not in _CACHE:
        _CACHE["nc"] = build()
    nc = _CACHE["nc"]
    in_maps = prep_inputs(posesglobal, waypointslocal, boundary,
                          boundarynormals)
    res = run_bass_kernel_spmd(nc, in_maps, list(range(NCORES)))
    total = 0.0
    for r in res.results:
        total += float(np.asarray(r["out"], dtype=np.float64).sum())
    return np.float32(total / (B * T))


# revision 34
# speedup vs baseline: 1.0875x; 1.0875x over previous
"""Trainium2 Bass kernel for nn_BoundaryLoss (retrieval 1-NN + boundary loss).

Math reformulation (validated against the reference on the fixed inputs):
rigid SE(3) transforms preserve distances and dot products, so the 1-NN
search and the signed-distance dot product can both be done in the GLOBAL
frame.  With wg = R_b @ w + t_b (waypoints to global frame, tiny host prep),
the per-(b,t) argmin over boundary points n of |w_local - p_local|^2 equals
argmax_n s'[n],  s'[n] = 2*wg.pg[n] - |pg[n]|^2,
and dots = (w_local - cp).cn = wg.ng[idx] - pg[idx].ng[idx].

Device pipeline per core (8-way data parallel over the 6400 (b,t) pairs),
per 128-waypoint tile (7 tiles per core):
  - PE: s'/8 via K=11 fp16 hi/lo split matmuls (a*b = ah*bh + ah*bl + al*bh
        per coordinate + 2 rows for p^2/8), fp32 PSUM accumulation; 4
        matmuls per 4-bank PSUM group, 2 groups ping-ponging.
  - ACT (+1 group on DVE): ONE 2048-wide PSUM->SBUF fp16 read per group.
        Multi-bank engine reads are safe because the Tile overlap tracker
        is bank-aware: no matmul writes those banks concurrently (the
        fatal case is PE-W || engine-R on the same bank).
  - DVE: one pairwise max of the halves (fp16 tensor_tensor), then ONE
        custom single-pass argmax instruction that pairs the quarters
        itself and scans them (ARGMAX_LAST_ANT: p = maxx(Src0, Src1);
        body = select(p == scan_max(p), Idx/16, -FLT_MAX), accum = MAX)
        over the 5120-wide quarter row.  Winner k gives the candidate
        set {k, k+5120, k+10240, k+15360}.
  - GPSIMD: four indirect-DMA gathers of [pg, p2, ng, pg.ng] rows per tile.
  - Tail (once, batched over [128, 7]): exact-fp32 rescore of the four
        candidates, select-tree pick, dots, exp_relu, mask; ones-matmul
        partition reduction -> [1, 7] per-core partial sums.
The scoring of tile j is software-pipelined with the argmax of tile j-1 so
DVE's evacuation copies never queue behind a scan, and the rhs table DMA is
split per group so the first matmul starts early.
Host: input prep/sharding + final sum of 8x7 partials / 6400.
Validated numerically on the fixed harness inputs: loss rel err 4.2e-05
(gate 2e-2).

HW notes (measured on the target cores): engine PSUM reads must not
overlap a concurrent matmul write to the same bank (Tile guards this);
matmul moving operand is capped at 512 fp32 PSUM columns (s3d3 ISA check
rejects 1024 for fp16); float32r matmul quantizes inputs to ~13 mantissa
bits (argmax-fatal); custom DVE ops stream at ~1.5 cyc/elem regardless of
spec depth; stock fp16 unit-stride tensor_tensor hits the 2x mode.
"""

import sys

sys.path.insert(0, "/opt/trn_rl_repo")

import numpy as np

from concourse import bacc, bass, mybir
import concourse.tile as tile
from concourse.bass_utils import run_bass_kernel_spmd
from concourse.dve_spec import (Spec, Src0, Src1, C2, MaxNeg, select, eq,
                                lower, AluOp, Idx, scan, maxx)
from concourse.dve_uop import DveOpSpec
import concourse.dve_ops as dve_ops
from concourse.dve_ops import DveOp

B, T, N = 64, 100, 20000
NCORES = 8
WPC = B * T // NCORES          # 800 waypoints per core
NTILES = 7                     # ceil(WPC / 128) partition tiles
CHUNK = 512                    # one PSUM bank of fp32
GRP = 2048                     # grouped evacuation width (4 banks)
NGRP = 10                      # groups per boundary row
DVE_GROUPS = (0, 1)            # groups evacuated by DVE at tile start,
                               # before the pipelined argmax occupies the
                               # engine (rest on ACT; measured optimum --
                               # any later DVE evac position regresses)
NCH = 40                       # 512-chunks per boundary row
NPAD = NCH * CHUNK             # 20480
SEG = NPAD // 2                # first pairing half width (10240)
QRT = NPAD // 4                # second pairing width (5120)
WPAD = NTILES * 128            # 896
KSPLIT = 11                    # fp16 split-matmul contraction rows

F32 = mybir.dt.float32
F16 = mybir.dt.float16
U32 = mybir.dt.uint32
U8 = mybir.dt.uint8
OP = mybir.AluOpType
AX = mybir.AxisListType
AF = mybir.ActivationFunctionType

# --- custom DVE op: single-pass last-tie-wins argmax (index scaled by 1/16
# so the fold stays exact even if the accumulator ran on post-cast fp16) ---
IDX_SCALE = 1.0 / 16.0
_p = maxx(Src0, Src1)
_ARGMAX_SPEC = Spec(body=select(eq(_p, scan(AluOp.MAX, _p)), Idx * C2, MaxNeg),
                    accum=AluOp.MAX)


def _register_argmax_op():
    name = "ARGMAX_LAST_ANT"
    for op in dve_ops.OPS:
        if op.name == name:
            return op

    def sha(ver):
        return DveOpSpec(name="tmp", opcode=1,
                         uops=lower(_ARGMAX_SPEC, ver=ver),
                         rd1_en=True).sha(ver)

    op = DveOp(name, _ARGMAX_SPEC, subdim=False,
               uops_sha={v: sha(v) for v in ("v3", "v4")})
    dve_ops.OPS.append(op)
    dve_ops.CUSTOM_DVE_SPECS[name] = _ARGMAX_SPEC
    row = max(dve_ops._SUB_OPCODE_FOR_NAME.values()) + 1
    assert row < 0x20
    dve_ops._SUB_OPCODE_FOR_NAME[name] = row
    return op


ARGMAX_LAST = _register_argmax_op()


def build(repeat=1):
    nc = bacc.Bacc("TRN2", target_bir_lowering=False, debug=False,
                   num_devices=NCORES)
    lhs = nc.dram_tensor("lhs", [KSPLIT, WPAD], F16, kind="ExternalInput").ap()
    rhs = nc.dram_tensor("rhs", [KSPLIT, NPAD], F16, kind="ExternalInput").ap()
    wgv = nc.dram_tensor("wgv", [128, NTILES, 3], F32, kind="ExternalInput").ap()
    msk = nc.dram_tensor("msk", [128, NTILES], F32, kind="ExternalInput").ap()
    tb8 = nc.dram_tensor("tb8", [NPAD, 8], F32, kind="ExternalInput").ap()
    out = nc.dram_tensor("out", [1, NTILES], F32, kind="ExternalOutput").ap()

    with tile.TileContext(nc) as tc:
        with (
            tc.tile_pool(name="const", bufs=1) as cpool,
            tc.tile_pool(name="s16p", bufs=2) as s16p,
            tc.tile_pool(name="mp", bufs=2) as mp,
            tc.tile_pool(name="sb", bufs=3) as sb,
            tc.tile_pool(name="ps", bufs=2, space="PSUM") as ps,
        ):
            lhs_sb = cpool.tile([KSPLIT, WPAD], F16)
            nc.sync.dma_start(out=lhs_sb[:], in_=lhs[:])
            rhs_sb = cpool.tile([KSPLIT, NPAD], F16)
            for g in range(NGRP):
                nc.sync.dma_start(out=rhs_sb[:, g * GRP:(g + 1) * GRP],
                                  in_=rhs[:, g * GRP:(g + 1) * GRP])
            wgv_sb = cpool.tile([128, NTILES, 3], F32)
            nc.sync.dma_start(out=wgv_sb[:], in_=wgv[:])
            msk_sb = cpool.tile([128, NTILES], F32)
            nc.sync.dma_start(out=msk_sb[:], in_=msk[:])
            ones_sb = cpool.tile([128, 1], F32)
            nc.vector.memset(ones_sb[:], 1.0)
            cand = cpool.tile([128, NTILES, 4, 8], F32)
            am_all = cpool.tile([128, NTILES, 2], F32)

            s16s = {}

            def emit_tile(j):
                s16 = s16p.tile([128, NPAD], F16, tag="s16")
                s16s[j] = s16
                for g in range(NGRP):
                    pg = ps.tile([128, GRP], F32, tag="mm")
                    for k in range(4):
                        c = 4 * g + k
                        nc.tensor.matmul(
                            out=pg[:, k * CHUNK:(k + 1) * CHUNK],
                            lhsT=lhs_sb[:, j * 128:(j + 1) * 128],
                            rhs=rhs_sb[:, c * CHUNK:(c + 1) * CHUNK],
                            start=True, stop=True,
                        )
                    dst = s16[:, g * GRP:(g + 1) * GRP]
                    if g in DVE_GROUPS:
                        nc.vector.tensor_copy(dst, pg[:])
                    else:
                        nc.scalar.activation(dst, pg[:], AF.Copy)
                    if g == NGRP // 2 - 1:
                        # half1 complete: its quarter-pair argmax can run
                        # during half2's scoring, inside DVE's idle window
                        m = mp.tile([128, QRT], F16, tag="m")
                        nc.vector._custom_dve(
                            ARGMAX_LAST, out=m[:], in0=s16[:, 0:QRT],
                            in1=s16[:, QRT:SEG], imm2=IDX_SCALE,
                            accum_out=am_all[:, j, 0:1])

            def emit_proc(j):
                # half2's quarter-pair argmax (half1's already ran inside
                # the tile's evac phase); candidates are the pair of each
                # half winner
                s16 = s16s.pop(j)
                m = mp.tile([128, QRT], F16, tag="m")
                nc.vector._custom_dve(
                    ARGMAX_LAST, out=m[:], in0=s16[:, SEG:SEG + QRT],
                    in1=s16[:, SEG + QRT:NPAD], imm2=IDX_SCALE,
                    accum_out=am_all[:, j, 1:2])
                idxu = sb.tile([128, 4], U32, tag="idxu")
                for c in range(4):
                    nc.vector.tensor_scalar(idxu[:, c:c + 1],
                                            am_all[:, j, c // 2:c // 2 + 1],
                                            1.0 / IDX_SCALE,
                                            float((c % 2) * QRT
                                                  + (c // 2) * SEG),
                                            OP.mult, OP.add)
                # gather [pg, p2, ng, pg.ng] rows for all four candidates
                for c in range(4):
                    nc.gpsimd.indirect_dma_start(
                        out=cand[:, j, c, :], out_offset=None, in_=tb8[:],
                        in_offset=bass.IndirectOffsetOnAxis(
                            ap=idxu[:, c:c + 1], axis=0),
                    )

            # software pipeline: tile j-1's argmax is emitted BEFORE tile
            # j's scoring so it runs on DVE while ACT drains the early
            # groups, and DVE's own (late-positioned) evacuation copies
            # are not queued behind the scan
            for j in range(NTILES * repeat):
                emit_tile(j % NTILES)
                if j > 0:
                    emit_proc((j - 1) % NTILES)
            emit_proc((NTILES * repeat - 1) % NTILES)

            # batched tail over [128, NTILES]: exact rescore, pick tree,
            # dots, exp_relu, mask
            sc = sb.tile([128, 4, NTILES], F32, tag="sc")
            dt = sb.tile([128, 4, NTILES], F32, tag="dt")
            t3 = sb.tile([128, NTILES, 3], F32, tag="t3")
            tr = sb.tile([128, NTILES], F32, tag="tr")
            for c in range(4):
                nc.vector.tensor_tensor(out=t3[:], in0=wgv_sb[:],
                                        in1=cand[:, :, c, 0:3], op=OP.mult)
                nc.vector.tensor_reduce(out=tr[:], in_=t3[:], axis=AX.X,
                                        op=OP.add)
                nc.vector.scalar_tensor_tensor(
                    out=sc[:, c, :], in0=tr[:], scalar=2.0,
                    in1=cand[:, :, c, 3], op0=OP.mult, op1=OP.subtract)
                nc.vector.tensor_tensor(out=t3[:], in0=wgv_sb[:],
                                        in1=cand[:, :, c, 4:7], op=OP.mult)
                nc.vector.tensor_reduce(out=tr[:], in_=t3[:], axis=AX.X,
                                        op=OP.add)
                nc.vector.tensor_tensor(out=dt[:, c, :], in0=tr[:],
                                        in1=cand[:, :, c, 7], op=OP.subtract)
            ge = sb.tile([128, NTILES], U8, tag="ge")
            sw = sb.tile([128, 2, NTILES], F32, tag="sw")
            dw = sb.tile([128, 2, NTILES], F32, tag="dw")
            for h in range(2):
                nc.vector.tensor_tensor(out=ge[:], in0=sc[:, 2 * h, :],
                                        in1=sc[:, 2 * h + 1, :], op=OP.is_ge)
                nc.vector.select(sw[:, h, :], ge[:], sc[:, 2 * h, :],
                                 sc[:, 2 * h + 1, :])
                nc.vector.select(dw[:, h, :], ge[:], dt[:, 2 * h, :],
                                 dt[:, 2 * h + 1, :])
            nc.vector.tensor_tensor(out=ge[:], in0=sw[:, 0, :],
                                    in1=sw[:, 1, :], op=OP.is_ge)
            dots = sb.tile([128, NTILES], F32, tag="dots")
            nc.vector.select(dots[:], ge[:], dw[:, 0, :], dw[:, 1, :])

            ecl = sb.tile([128, NTILES], F32, tag="ecl")
            nc.vector.tensor_scalar_min(ecl[:], dots[:], 0.0)
            ex = sb.tile([128, NTILES], F32, tag="ex")
            nc.scalar.activation(ex[:], ecl[:], AF.Exp, scale=0.5)
            p1 = sb.tile([128, NTILES], F32, tag="p1")
            nc.vector.tensor_scalar_add(p1[:], dots[:], 1.0)
            gt = sb.tile([128, NTILES], U8, tag="gt")
            nc.vector.tensor_scalar(gt[:], dots[:], 0.0, None, OP.is_gt)
            er = sb.tile([128, NTILES], F32, tag="er")
            nc.vector.select(er[:], gt[:], p1[:], ex[:])
            erm = sb.tile([128, NTILES], F32, tag="erm")
            nc.vector.tensor_tensor(out=erm[:], in0=er[:], in1=msk_sb[:],
                                    op=OP.mult)

            po = ps.tile([1, NTILES], F32, tag="mm")
            nc.tensor.matmul(out=po[:], lhsT=ones_sb[:, 0:1], rhs=erm[:],
                             start=True, stop=True)
            ob = sb.tile([1, NTILES], F32, tag="ob")
            nc.vector.tensor_copy(ob[:], po[:])
            nc.sync.dma_start(out=out[:], in_=ob[:])

    nc.compile()
    return nc


def _f16_split(x32):
    hi = x32.astype(np.float16)
    lo = (x32 - hi.astype(np.float32)).astype(np.float16)
    return hi, lo


def prep_inputs(posesglobal, waypointslocal, boundary, boundarynormals):
    poses = np.asarray(posesglobal, dtype=np.float32)
    wpts = np.asarray(waypointslocal, dtype=np.float32)
    bound = np.asarray(boundary, dtype=np.float32)
    nrm = np.asarray(boundarynormals, dtype=np.float32)

    R = poses[:, :3, :3]
    t = poses[:, :3, 3]
    wg = (np.einsum("bij,btj->bti", R, wpts).astype(np.float32)
          + t[:, None, :]).astype(np.float32).reshape(-1, 3)   # [B*T, 3]

    pg = bound[:3]
    p2 = (pg[0] * pg[0] + pg[1] * pg[1] + pg[2] * pg[2]).astype(np.float32)
    pn = (pg[0] * nrm[0] + pg[1] * nrm[1] + pg[2] * nrm[2]).astype(np.float32)

    # rhs rows: per coord d -> [bh_d, bl_d, bh_d]; then [ch, cl] for p2/8
    bh, bl = _f16_split(pg)                     # [3, N] each
    ch, cl = _f16_split(p2 / 8.0)
    rhs = np.zeros((KSPLIT, NPAD), np.float16)
    for d in range(3):
        rhs[3 * d + 0, :N] = bh[d]
        rhs[3 * d + 1, :N] = bl[d]
        rhs[3 * d + 2, :N] = bh[d]
    rhs[9, :N] = ch
    rhs[10, :N] = cl
    rhs[9, N:] = np.float16(60000.0)   # pad columns can never win the argmax

    # combined gather table [pg, p2, ng, pg.ng]; pad rows lose the exact
    # rescore via p2 = 1e30 (candidate 1 may index into the pad range)
    tb8 = np.zeros((NPAD, 8), np.float32)
    tb8[:N, 0:3] = pg.T
    tb8[:N, 3] = p2
    tb8[:N, 4:7] = nrm.T
    tb8[:N, 7] = pn
    tb8[N:, 3] = 1.0e30

    valid = (np.arange(WPAD) < WPC)
    msk = valid.reshape(NTILES, 128).T.astype(np.float32).copy()  # [128, 7]

    in_maps = []
    for c in range(NCORES):
        w = wg[c * WPC:(c + 1) * WPC]
        wp = np.zeros((WPAD, 3), np.float32)
        wp[:WPC] = w
        ah, al = _f16_split(wp.T / 4.0)          # [3, WPAD] each (= 2*wg/8)
        lhs = np.zeros((KSPLIT, WPAD), np.float16)
        for d in range(3):
            lhs[3 * d + 0] = ah[d]
            lhs[3 * d + 1] = ah[d]
            lhs[3 * d + 2] = al[d]
        lhs[9] = np.float16(-1.0)
        lhs[10] = np.float16(-1.0)
        wgv = wp.reshape(NTILES, 128, 3).transpose(1, 0, 2).copy()
        in_maps.append({"lhs": lhs, "rhs": rhs, "wgv": wgv,
                        "msk": msk, "tb8": tb8})
    return in_maps


_CACHE = {}


def kernel(posesglobal, waypointslocal, boundary, boundarynormals):
    if "nc" not in _CACHE:
        _CACHE["nc"] = build()
    nc = _CACHE["nc"]
    in_maps = prep_inputs(posesglobal, waypointslocal, boundary,
                          boundarynormals)
    res = run_bass_kernel_spmd(nc, in_maps, list(range(NCORES)))
    total = 0.0
    for r in res.results:
        total += float(np.asarray(r["out"], dtype=np.float64).sum())
    return np.float32(total / (B * T))


# revision 35
# speedup vs baseline: 1.1057x; 1.0167x over previous
"""Trainium2 Bass kernel for nn_BoundaryLoss (retrieval 1-NN + boundary loss).

Math reformulation (validated against the reference on the fixed inputs):
rigid SE(3) transforms preserve distances and dot products, so the 1-NN
search and the signed-distance dot product can both be done in the GLOBAL
frame.  With wg = R_b @ w + t_b (waypoints to global frame, tiny host prep),
the per-(b,t) argmin over boundary points n of |w_local - p_local|^2 equals
argmax_n s'[n],  s'[n] = 2*wg.pg[n] - |pg[n]|^2,
and dots = (w_local - cp).cn = wg.ng[idx] - pg[idx].ng[idx].

Device pipeline per core (8-way data parallel over the 6400 (b,t) pairs),
per 128-waypoint tile (7 tiles per core):
  - PE: s'/8 via K=11 fp16 hi/lo split matmuls (a*b = ah*bh + ah*bl + al*bh
        per coordinate + 2 rows for p^2/8), fp32 PSUM accumulation; 4
        matmuls per 4-bank PSUM group, 2 groups ping-ponging.
  - ACT (+1 group on DVE): ONE 2048-wide PSUM->SBUF fp16 read per group.
        Multi-bank engine reads are safe because the Tile overlap tracker
        is bank-aware: no matmul writes those banks concurrently (the
        fatal case is PE-W || engine-R on the same bank).
  - DVE: one pairwise max of the halves (fp16 tensor_tensor), then ONE
        custom single-pass argmax instruction that pairs the quarters
        itself and scans them (ARGMAX_LAST_ANT: p = maxx(Src0, Src1);
        body = select(p == scan_max(p), Idx/16, -FLT_MAX), accum = MAX)
        over the 5120-wide quarter row.  Winner k gives the candidate
        set {k, k+5120, k+10240, k+15360}.
  - GPSIMD: four indirect-DMA gathers of [pg, p2, ng, pg.ng] rows per tile.
  - Tail (once, batched over [128, 7]): exact-fp32 rescore of the four
        candidates, select-tree pick, dots, exp_relu, mask; ones-matmul
        partition reduction -> [1, 7] per-core partial sums.
The scoring of tile j is software-pipelined with the argmax of tile j-1 so
DVE's evacuation copies never queue behind a scan, and the rhs table DMA is
split per group so the first matmul starts early.
Host: input prep/sharding + final sum of 8x7 partials / 6400.
Validated numerically on the fixed harness inputs: loss rel err 4.2e-05
(gate 2e-2).

HW notes (measured on the target cores): engine PSUM reads must not
overlap a concurrent matmul write to the same bank (Tile guards this);
matmul moving operand is capped at 512 fp32 PSUM columns (s3d3 ISA check
rejects 1024 for fp16); float32r matmul quantizes inputs to ~13 mantissa
bits (argmax-fatal); custom DVE ops stream at ~1.5 cyc/elem regardless of
spec depth; stock fp16 unit-stride tensor_tensor hits the 2x mode.
"""

import sys

sys.path.insert(0, "/opt/trn_rl_repo")

import numpy as np

from concourse import bacc, bass, mybir
import concourse.tile as tile
from concourse.bass_utils import run_bass_kernel_spmd
from concourse.dve_spec import (Spec, Src0, Src1, C2, MaxNeg, select, eq,
                                lower, AluOp, Idx, scan, maxx)
from concourse.dve_uop import DveOpSpec
import concourse.dve_ops as dve_ops
from concourse.dve_ops import DveOp

B, T, N = 64, 100, 20000
NCORES = 8
WPC = B * T // NCORES          # 800 waypoints per core
NTILES = 7                     # ceil(WPC / 128) partition tiles
CHUNK = 512                    # one PSUM bank of fp32
GRP = 2048                     # grouped evacuation width (4 banks)
NGRP = 10                      # groups per boundary row
DVE_GROUPS = (0, 1)            # groups evacuated by DVE at tile start,
                               # before the pipelined argmax occupies the
                               # engine (rest on ACT; measured optimum --
                               # any later DVE evac position regresses)
NCH = 40                       # 512-chunks per boundary row
NPAD = NCH * CHUNK             # 20480
SEG = NPAD // 2                # first pairing half width (10240)
QRT = NPAD // 4                # second pairing width (5120)
WPAD = NTILES * 128            # 896
KSPLIT = 11                    # fp16 split-matmul contraction rows

F32 = mybir.dt.float32
F16 = mybir.dt.float16
U32 = mybir.dt.uint32
U8 = mybir.dt.uint8
OP = mybir.AluOpType
AX = mybir.AxisListType
AF = mybir.ActivationFunctionType

# --- custom DVE op: single-pass last-tie-wins argmax (index scaled by 1/16
# so the fold stays exact even if the accumulator ran on post-cast fp16) ---
IDX_SCALE = 1.0 / 16.0
_p = maxx(Src0, Src1)
_ARGMAX_SPEC = Spec(body=select(eq(_p, scan(AluOp.MAX, _p)), Idx * C2, MaxNeg),
                    accum=AluOp.MAX)


def _register_argmax_op():
    name = "ARGMAX_LAST_ANT"
    for op in dve_ops.OPS:
        if op.name == name:
            return op

    def sha(ver):
        return DveOpSpec(name="tmp", opcode=1,
                         uops=lower(_ARGMAX_SPEC, ver=ver),
                         rd1_en=True).sha(ver)

    op = DveOp(name, _ARGMAX_SPEC, subdim=False,
               uops_sha={v: sha(v) for v in ("v3", "v4")})
    dve_ops.OPS.append(op)
    dve_ops.CUSTOM_DVE_SPECS[name] = _ARGMAX_SPEC
    row = max(dve_ops._SUB_OPCODE_FOR_NAME.values()) + 1
    assert row < 0x20
    dve_ops._SUB_OPCODE_FOR_NAME[name] = row
    return op


ARGMAX_LAST = _register_argmax_op()


def build(repeat=1):
    nc = bacc.Bacc("TRN2", target_bir_lowering=False, debug=False,
                   num_devices=NCORES)
    lhs = nc.dram_tensor("lhs", [KSPLIT, WPAD], F16, kind="ExternalInput").ap()
    rhs = nc.dram_tensor("rhs", [KSPLIT, NPAD], F16, kind="ExternalInput").ap()
    wgv = nc.dram_tensor("wgv", [128, NTILES, 3], F32, kind="ExternalInput").ap()
    msk = nc.dram_tensor("msk", [128, NTILES], F32, kind="ExternalInput").ap()
    tb8 = nc.dram_tensor("tb8", [NPAD, 8], F32, kind="ExternalInput").ap()
    out = nc.dram_tensor("out", [1, NTILES], F32, kind="ExternalOutput").ap()

    with tile.TileContext(nc) as tc:
        with (
            tc.tile_pool(name="const", bufs=1) as cpool,
            tc.tile_pool(name="s16p", bufs=2) as s16p,
            tc.tile_pool(name="mp", bufs=2) as mp,
            tc.tile_pool(name="sb", bufs=3) as sb,
            tc.tile_pool(name="ps", bufs=2, space="PSUM") as ps,
        ):
            lhs_sb = cpool.tile([KSPLIT, WPAD], F16)
            nc.sync.dma_start(out=lhs_sb[:], in_=lhs[:])
            rhs_sb = cpool.tile([KSPLIT, NPAD], F16)
            for g in range(NGRP):
                nc.sync.dma_start(out=rhs_sb[:, g * GRP:(g + 1) * GRP],
                                  in_=rhs[:, g * GRP:(g + 1) * GRP])
            wgv_sb = cpool.tile([128, NTILES, 3], F32)
            nc.sync.dma_start(out=wgv_sb[:], in_=wgv[:])
            msk_sb = cpool.tile([128, NTILES], F32)
            nc.sync.dma_start(out=msk_sb[:], in_=msk[:])
            ones_sb = cpool.tile([128, 1], F32)
            nc.vector.memset(ones_sb[:], 1.0)
            cand = cpool.tile([128, NTILES, 4, 8], F32)
            am_all = cpool.tile([128, NTILES], F32)

            s16s = {}

            def emit_tile(j):
                s16 = s16p.tile([128, NPAD], F16, tag="s16")
                s16s[j] = s16
                for g in range(NGRP):
                    pg = ps.tile([128, GRP], F32, tag="mm")
                    for k in range(4):
                        c = 4 * g + k
                        nc.tensor.matmul(
                            out=pg[:, k * CHUNK:(k + 1) * CHUNK],
                            lhsT=lhs_sb[:, j * 128:(j + 1) * 128],
                            rhs=rhs_sb[:, c * CHUNK:(c + 1) * CHUNK],
                            start=True, stop=True,
                        )
                    dst = s16[:, g * GRP:(g + 1) * GRP]
                    if g in DVE_GROUPS:
                        nc.vector.tensor_copy(dst, pg[:])
                    else:
                        nc.scalar.activation(dst, pg[:], AF.Copy)

            def emit_proc(j):
                # two fp16 2x pairing rounds, then one-pass argmax over 5120
                s16 = s16s.pop(j)
                m = mp.tile([128, SEG], F16, tag="m")
                nc.vector.tensor_tensor(out=m[:], in0=s16[:, 0:SEG],
                                        in1=s16[:, SEG:NPAD], op=OP.max)
                # argmax op pairs the quarters itself (maxx(Src0, Src1))
                nc.vector._custom_dve(ARGMAX_LAST, out=m[:, 0:QRT],
                                      in0=m[:, 0:QRT], in1=m[:, QRT:SEG],
                                      imm2=IDX_SCALE,
                                      accum_out=am_all[:, j:j + 1])
                idxu = sb.tile([128, 4], U32, tag="idxu")
                for c in range(4):
                    nc.vector.tensor_scalar(idxu[:, c:c + 1],
                                            am_all[:, j:j + 1],
                                            1.0 / IDX_SCALE, float(c * QRT),
                                            OP.mult, OP.add)
                # gather [pg, p2, ng, pg.ng] rows for all four candidates
                for c in range(4):
                    nc.gpsimd.indirect_dma_start(
                        out=cand[:, j, c, :], out_offset=None, in_=tb8[:],
                        in_offset=bass.IndirectOffsetOnAxis(
                            ap=idxu[:, c:c + 1], axis=0),
                    )

            # software pipeline: tile j-1's argmax is emitted BEFORE tile
            # j's scoring so it runs on DVE while ACT drains the early
            # groups, and DVE's own (late-positioned) evacuation copies
            # are not queued behind the scan
            for j in range(NTILES * repeat):
                emit_tile(j % NTILES)
                if j > 0:
                    emit_proc((j - 1) % NTILES)
            emit_proc((NTILES * repeat - 1) % NTILES)

            # batched tail over [128, NTILES]: exact rescore, pick tree,
            # dots, exp_relu, mask
            sc = sb.tile([128, 4, NTILES], F32, tag="sc")
            dt = sb.tile([128, 4, NTILES], F32, tag="dt")
            t3 = sb.tile([128, NTILES, 3], F32, tag="t3")
            tr = sb.tile([128, NTILES], F32, tag="tr")
            for c in range(4):
                nc.vector.tensor_tensor(out=t3[:], in0=wgv_sb[:],
                                        in1=cand[:, :, c, 0:3], op=OP.mult)
                nc.vector.tensor_reduce(out=tr[:], in_=t3[:], axis=AX.X,
                                        op=OP.add)
                nc.vector.scalar_tensor_tensor(
                    out=sc[:, c, :], in0=tr[:], scalar=2.0,
                    in1=cand[:, :, c, 3], op0=OP.mult, op1=OP.subtract)
                nc.vector.tensor_tensor(out=t3[:], in0=wgv_sb[:],
                                        in1=cand[:, :, c, 4:7], op=OP.mult)
                nc.vector.tensor_reduce(out=tr[:], in_=t3[:], axis=AX.X,
                                        op=OP.add)
                nc.vector.tensor_tensor(out=dt[:, c, :], in0=tr[:],
                                        in1=cand[:, :, c, 7], op=OP.subtract)
            ge = sb.tile([128, NTILES], U8, tag="ge")
            sw = sb.tile([128, 2, NTILES], F32, tag="sw")
            dw = sb.tile([128, 2, NTILES], F32, tag="dw")
            for h in range(2):
                nc.vector.tensor_tensor(out=ge[:], in0=sc[:, 2 * h, :],
                                        in1=sc[:, 2 * h + 1, :], op=OP.is_ge)
                nc.vector.select(sw[:, h, :], ge[:], sc[:, 2 * h, :],
                                 sc[:, 2 * h + 1, :])
                nc.vector.select(dw[:, h, :], ge[:], dt[:, 2 * h, :],
                                 dt[:, 2 * h + 1, :])
            nc.vector.tensor_tensor(out=ge[:], in0=sw[:, 0, :],
                                    in1=sw[:, 1, :], op=OP.is_ge)
            dots = sb.tile([128, NTILES], F32, tag="dots")
            nc.vector.select(dots[:], ge[:], dw[:, 0, :], dw[:, 1, :])

            ecl = sb.tile([128, NTILES], F32, tag="ecl")
            nc.vector.tensor_scalar_min(ecl[:], dots[:], 0.0)
            ex = sb.tile([128, NTILES], F32, tag="ex")
            nc.scalar.activation(ex[:], ecl[:], AF.Exp, scale=0.5)
            p1 = sb.tile([128, NTILES], F32, tag="p1")
            nc.vector.tensor_scalar_add(p1[:], dots[:], 1.0)
            gt = sb.tile([128, NTILES], U8, tag="gt")
            nc.vector.tensor_scalar(gt[:], dots[:], 0.0, None, OP.is_gt)
            er = sb.tile([128, NTILES], F32, tag="er")
            nc.vector.select(er[:], gt[:], p1[:], ex[:])
            erm = sb.tile([128, NTILES], F32, tag="erm")
            nc.vector.tensor_tensor(out=erm[:], in0=er[:], in1=msk_sb[:],
                                    op=OP.mult)

            po = ps.tile([1, NTILES], F32, tag="mm")
            nc.tensor.matmul(out=po[:], lhsT=ones_sb[:, 0:1], rhs=erm[:],
                             start=True, stop=True)
            ob = sb.tile([1, NTILES], F32, tag="ob")
            nc.vector.tensor_copy(ob[:], po[:])
            nc.sync.dma_start(out=out[:], in_=ob[:])

    nc.compile()
    return nc


def _f16_split(x32):
    hi = x32.astype(np.float16)
    lo = (x32 - hi.astype(np.float32)).astype(np.float16)
    return hi, lo


def prep_inputs(posesglobal, waypointslocal, boundary, boundarynormals):
    poses = np.asarray(posesglobal, dtype=np.float32)
    wpts = np.asarray(waypointslocal, dtype=np.float32)
    bound = np.asarray(boundary, dtype=np.float32)
    nrm = np.asarray(boundarynormals, dtype=np.float32)

    R = poses[:, :3, :3]
    t = poses[:, :3, 3]
    wg = (np.einsum("bij,btj->bti", R, wpts).astype(np.float32)
          + t[:, None, :]).astype(np.float32).reshape(-1, 3)   # [B*T, 3]

    pg = bound[:3]
    p2 = (pg[0] * pg[0] + pg[1] * pg[1] + pg[2] * pg[2]).astype(np.float32)
    pn = (pg[0] * nrm[0] + pg[1] * nrm[1] + pg[2] * nrm[2]).astype(np.float32)

    # rhs rows: per coord d -> [bh_d, bl_d, bh_d]; then [ch, cl] for p2/8
    bh, bl = _f16_split(pg)                     # [3, N] each
    ch, cl = _f16_split(p2 / 8.0)
    rhs = np.zeros((KSPLIT, NPAD), np.float16)
    for d in range(3):
        rhs[3 * d + 0, :N] = bh[d]
        rhs[3 * d + 1, :N] = bl[d]
        rhs[3 * d + 2, :N] = bh[d]
    rhs[9, :N] = ch
    rhs[10, :N] = cl
    rhs[9, N:] = np.float16(60000.0)   # pad columns can never win the argmax

    # combined gather table [pg, p2, ng, pg.ng]; pad rows lose the exact
    # rescore via p2 = 1e30 (candidate 1 may index into the pad range)
    tb8 = np.zeros((NPAD, 8), np.float32)
    tb8[:N, 0:3] = pg.T
    tb8[:N, 3] = p2
    tb8[:N, 4:7] = nrm.T
    tb8[:N, 7] = pn
    tb8[N:, 3] = 1.0e30

    valid = (np.arange(WPAD) < WPC)
    msk = valid.reshape(NTILES, 128).T.astype(np.float32).copy()  # [128, 7]

    in_maps = []
    for c in range(NCORES):
        w = wg[c * WPC:(c + 1) * WPC]
        wp = np.zeros((WPAD, 3), np.float32)
        wp[:WPC] = w
        ah, al = _f16_split(wp.T / 4.0)          # [3, WPAD] each (= 2*wg/8)
        lhs = np.zeros((KSPLIT, WPAD), np.float16)
        for d in range(3):
            lhs[3 * d + 0] = ah[d]
            lhs[3 * d + 1] = ah[d]
            lhs[3 * d + 2] = al[d]
        lhs[9] = np.float16(-1.0)
        lhs[10] = np.float16(-1.0)
        wgv = wp.reshape(NTILES, 128, 3).transpose(1, 0, 2).copy()
        in_maps.append({"lhs": lhs, "rhs": rhs, "wgv": wgv,
                        "msk": msk, "tb8": tb8})
    return in_maps


_CACHE = {}


def kernel(posesglobal, waypointslocal, boundary, boundarynormals):
    if "nc" not in _CACHE:
        _CACHE["nc"] = build()
    nc = _CACHE["nc"]
    in_maps = prep_inputs(posesglobal, waypointslocal, boundary,
                          boundarynormals)
    res = run_bass_kernel_spmd(nc, in_maps, list(range(NCORES)))
    total = 0.0
    for r in res.results:
        total += float(np.asarray(r["out"], dtype=np.float64).sum())
    return np.float32(total / (B * T))
